# revision 1
# baseline (speedup 1.0000x reference)
"""Trainium2 Bass kernel for nn_BasicLayer (sparse cluster attention, 2 layers).

Strategy
--------
The scanline (boustrophedon) ordering commutes with every per-token op in the
network (LN, matmuls, residuals) and the attention operates on contiguous
64-token clusters *of the ordered sequence*.  So: gather x into curve order
once on the host, run both transformer layers on-device with attention over
contiguous clusters, and scatter back at the end.

Sharding: 65536 tokens total (4 batches x 16384) -> 8192 tokens per core
(half a batch each, aligned to cluster boundaries).  Weights replicated.

On-device layout: token-major fp32 residual stream (features padded 192->256
so layout flips are full 128x128 DMA-xbar transposes); bf16 feature-major
matmul operands.  LN affine and all biases are folded on the host.  Attention
runs per 128-token cluster-pair: per-head scores/O matmuls are K=128/N=128
full-pair blocks; cross-cluster leakage is killed by keeping the off-diagonal
blocks of the softmax matrix P permanently zero.
"""

import os
import numpy as np
import ml_dtypes

# ---- problem constants (hardcoded per contract) ----
B, N, D = 4, 16384, 192
DP = 256                             # padded feature dim for the residual
HEADS, DH, CLM = 6, 32, 64
GRID_W = 128
DEPTH = 2
NCORES = 8
T = (B * N) // NCORES                # 8192 tokens per core
SUB = 128
NSUB = 4
TILE = SUB * NSUB                    # 512-token supertile = 4 cluster pairs
NTILES = T // TILE                   # 16
DFF = 768

_COMPILED = {}


def _scanline_order(pos, w):
    ix = np.floor(pos[..., 0]).astype(np.int64)
    iy = np.floor(pos[..., 1]).astype(np.int64)
    key = iy * w + np.where(iy % 2 == 1, w - 1 - ix, ix)
    return np.argsort(key, axis=1, kind="stable")


def _fold_weights(inputs):
    """Fold LN affine + biases into matmul weights. Returns per-layer dicts of
    numpy arrays laid out exactly as the DRAM tensors the kernel declares."""
    bf16 = ml_dtypes.bfloat16
    scale = DH ** -0.5
    layers = []
    for i in range(DEPTH):
        g1 = np.asarray(inputs["ln1_g"][i], np.float64)
        b1 = np.asarray(inputs["ln1_b"][i], np.float64)
        Wqkv = np.asarray(inputs["w_qkv"][i], np.float64)
        bqkv = np.asarray(inputs["b_qkv"][i], np.float64)
        w_eff = g1[:, None] * Wqkv
        b_eff = b1 @ Wqkv + bqkv
        wq = w_eff[:, 0:D] * scale
        bq = b_eff[0:D] * scale
        wk = w_eff[:, D:2 * D]
        bk = b_eff[D:2 * D]
        wv = w_eff[:, 2 * D:3 * D]
        bv = b_eff[2 * D:3 * D]
        # qk weight M-layout: [q h0-3 | k h0-3 | q h4-5 | k h4-5]
        wqk = np.concatenate(
            [wq[:, :128], wk[:, :128], wq[:, 128:], wk[:, 128:]], axis=1)
        pad64 = np.zeros(64)
        bqk = np.stack(
            [bq[:128], bk[:128],
             np.concatenate([bq[128:], pad64]),
             np.concatenate([bk[128:], pad64])], axis=1)
        wp = np.asarray(inputs["w_proj"][i], np.float64)
        bp = np.asarray(inputs["b_proj"][i], np.float64)
        g2 = np.asarray(inputs["ln2_g"][i], np.float64)
        b2 = np.asarray(inputs["ln2_b"][i], np.float64)
        W1 = np.asarray(inputs["w_fc1"][i], np.float64)
        w1_eff = g2[:, None] * W1
        b1_eff = b2 @ W1 + np.asarray(inputs["b_fc1"][i], np.float64)
        W2 = np.asarray(inputs["w_fc2"][i], np.float64)
        bfc2 = np.asarray(inputs["b_fc2"][i], np.float64)
        bv_t = np.stack(
            [bv[:128], np.concatenate([bv[128:], np.zeros(64)])], axis=1)
        layers.append({
            f"wqk{i}": wqk.astype(bf16),
            f"bqk{i}": bqk.astype(np.float32),
            f"wv{i}": wv.astype(bf16),
            f"bv{i}": bv_t.astype(np.float32),
            f"wp{i}": wp.astype(bf16),
            f"bp{i}": np.tile(bp.astype(np.float32), (128, 1)),
            f"w1{i}": w1_eff.astype(bf16),
            f"b1{i}": b1_eff.reshape(6, 128).T.copy().astype(np.float32),
            f"w2{i}": W2.astype(bf16),
            f"b2{i}": np.tile(bfc2.astype(np.float32), (128, 1)),
        })
    return layers


def _build_nc(biases_zero=False):
    """Build + compile the Bass program (once per process)."""
    key = ("nc", biases_zero)
    if key in _COMPILED:
        return _COMPILED[key]

    from contextlib import ExitStack
    import concourse.bass as bass
    import concourse.tile as tile
    from concourse import bacc, mybir
    from concourse.bass import ts, ds

    f32 = mybir.dt.float32
    bf16 = mybir.dt.bfloat16
    AF = mybir.ActivationFunctionType
    OP = mybir.AluOpType

    nc = bacc.Bacc("TRN2", target_bir_lowering=False, debug=False,
                   enable_asserts=False, num_devices=NCORES)

    x_d = nc.dram_tensor("x", [T, DP], f32, kind="ExternalInput").ap()
    y_d = nc.dram_tensor("y", [T, D], f32, kind="ExternalOutput").ap()
    wd = []
    for i in range(DEPTH):
        wd.append({
            "wqk": nc.dram_tensor(f"wqk{i}", [D, 384], bf16, kind="ExternalInput").ap(),
            "bqk": nc.dram_tensor(f"bqk{i}", [128, 4], f32, kind="ExternalInput").ap(),
            "wv": nc.dram_tensor(f"wv{i}", [D, D], bf16, kind="ExternalInput").ap(),
            "bv": nc.dram_tensor(f"bv{i}", [128, 2], f32, kind="ExternalInput").ap(),
            "wp": nc.dram_tensor(f"wp{i}", [D, D], bf16, kind="ExternalInput").ap(),
            "bp": nc.dram_tensor(f"bp{i}", [128, D], f32, kind="ExternalInput").ap(),
            "w1": nc.dram_tensor(f"w1{i}", [D, DFF], bf16, kind="ExternalInput").ap(),
            "b1": nc.dram_tensor(f"b1{i}", [128, 6], f32, kind="ExternalInput").ap(),
            "w2": nc.dram_tensor(f"w2{i}", [DFF, D], bf16, kind="ExternalInput").ap(),
            "b2": nc.dram_tensor(f"b2{i}", [128, D], f32, kind="ExternalInput").ap(),
        })

    with tile.TileContext(nc) as tc, ExitStack() as ctx:
        consts = ctx.enter_context(tc.tile_pool(name="consts", bufs=1))
        xpool = ctx.enter_context(tc.tile_pool(name="xpool", bufs=3))
        wpool = ctx.enter_context(tc.tile_pool(name="wpool", bufs=3))
        spool = ctx.enter_context(tc.tile_pool(name="spool", bufs=3))
        stpool = ctx.enter_context(tc.tile_pool(name="stpool", bufs=6))
        pp_early = ctx.enter_context(tc.tile_pool(name="pp_early", bufs=2, space="PSUM"))
        pp_mid = ctx.enter_context(tc.tile_pool(name="pp_mid", bufs=4, space="PSUM"))
        pp_late = ctx.enter_context(tc.tile_pool(name="pp_late", bufs=2, space="PSUM"))

        from concourse.masks import make_identity
        ident = consts.tile([128, 128], bf16)
        make_identity(nc, ident)
        eps_t = consts.tile([128, 1], f32)
        nc.vector.memset(eps_t, 1e-5)
        # persistent softmax tiles: off-diagonal (cross-cluster) blocks stay 0
        NPBUF = 3
        p_bufs = []
        for pb_i in range(NPBUF):
            pb = consts.tile([128, HEADS, 128], bf16, name=f"pbuf{pb_i}")
            nc.vector.memset(pb, 0.0)
            p_bufs.append(pb)

        # --- load weights into SBUF once ---
        W = []
        for i in range(DEPTH):
            d = wd[i]
            sb = {}
            sb["wqk0"] = consts.tile([128, 384], bf16, name=f"wqk0{i}")
            sb["wqk1"] = consts.tile([64, 384], bf16, name=f"wqk1{i}")
            nc.sync.dma_start(out=sb["wqk0"], in_=d["wqk"][0:128])
            nc.sync.dma_start(out=sb["wqk1"], in_=d["wqk"][128:192])
            sb["wv0"] = consts.tile([128, D], bf16, name=f"wv0{i}")
            sb["wv1"] = consts.tile([64, D], bf16, name=f"wv1{i}")
            nc.sync.dma_start(out=sb["wv0"], in_=d["wv"][0:128])
            nc.sync.dma_start(out=sb["wv1"], in_=d["wv"][128:192])
            sb["wp0"] = consts.tile([128, D], bf16, name=f"wp0{i}")
            sb["wp1"] = consts.tile([64, D], bf16, name=f"wp1{i}")
            nc.sync.dma_start(out=sb["wp0"], in_=d["wp"][0:128])
            nc.sync.dma_start(out=sb["wp1"], in_=d["wp"][128:192])
            sb["w10"] = consts.tile([128, DFF], bf16, name=f"w10{i}")
            sb["w11"] = consts.tile([64, DFF], bf16, name=f"w11{i}")
            nc.sync.dma_start(out=sb["w10"], in_=d["w1"][0:128])
            nc.sync.dma_start(out=sb["w11"], in_=d["w1"][128:192])
            sb["w2m"] = consts.tile([128, 6, D], bf16, name=f"w2m{i}")
            nc.sync.dma_start(
                out=sb["w2m"],
                in_=d["w2"].rearrange("(m p) n -> p m n", p=128))
            for nm in ("bqk", "bv", "b1", "bp", "b2"):
                shp = {"bqk": [128, 4], "bv": [128, 2], "b1": [128, 6],
                       "bp": [128, D], "b2": [128, D]}[nm]
                sb[nm] = consts.tile(shp, f32, name=f"{nm}{i}")
                nc.sync.dma_start(out=sb[nm], in_=d[nm])
            W.append(sb)

        pair_ctr = [0]

        def layernorm_to_fm(x_t, tag):
            """LN on token-major x_t -> bf16 feature-major chunk tiles
            fmA (feats 0:128) and fmA2 (feats 128:256; rows 64: are pad)."""
            xn = wpool.tile([128, NSUB, DP], bf16, tag=f"xn{tag}", name=f"xn{tag}")
            mv4 = stpool.tile([128, NSUB, 2], f32, tag="mv", name="mv4")
            sd4 = stpool.tile([128, NSUB], f32, tag="sd", name="sd4")
            r4 = stpool.tile([128, NSUB], f32, tag="r", name="r4")
            for s in range(NSUB):
                st = stpool.tile([128, 6], f32, tag="st", name="st")
                nc.vector.bn_stats(st, x_t[:, s, 0:D])
                nc.vector.bn_aggr(mv4[:, s], st)
            nc.scalar.activation(sd4, mv4[:, :, 1], AF.Sqrt, bias=eps_t)
            nc.vector.reciprocal(r4, sd4)
            fmA = wpool.tile([128, TILE], bf16, tag=f"fmA{tag}", name=f"fmA{tag}")
            fmA2 = wpool.tile([128, TILE], bf16, tag=f"fmA2{tag}", name=f"fmA2{tag}")
            psA = pp_early.tile([128, TILE], bf16, tag="early", name="psA")
            psA2 = pp_early.tile([128, TILE], bf16, tag="early", name="psA2")
            for s in range(NSUB):
                lnt = stpool.tile([128, DP], f32, tag="lnt", name="lnt")
                mus = mv4[:, s, 0:1]
                mu_b = bass.AP(tensor=mus.tensor, offset=mus.offset,
                               ap=[mus.ap[0], [0, DP]])
                rs = r4[:, s:s + 1]
                r_b = bass.AP(tensor=rs.tensor, offset=rs.offset,
                              ap=[rs.ap[0], [0, DP]])
                nc.gpsimd.tensor_tensor(out=lnt, in0=x_t[:, s], in1=mu_b,
                                        op=OP.subtract)
                nc.gpsimd.tensor_tensor(out=xn[:, s], in0=lnt, in1=r_b,
                                        op=OP.mult)
                nc.tensor.transpose(psA[:, ts(s, 128)], xn[:, s, 0:128], ident)
                nc.tensor.transpose(psA2[:, ts(s, 128)], xn[:, s, 128:256], ident)
            nc.vector.tensor_copy(fmA, psA)
            nc.vector.tensor_copy(fmA2, psA2)
            return fmA, fmA2

        def mlp_block(sb, ynA, ynA2, x_t):
            hfm = wpool.tile([128, 6, TILE], bf16, tag="hfm", name="hfm")
            for m in range(6):
                ps = pp_late.tile([128, TILE], f32, tag="late", name="psh")
                nc.tensor.matmul(ps, sb["w10"][:, ts(m, 128)], ynA,
                                 start=True, stop=False)
                nc.tensor.matmul(ps, sb["w11"][:, ts(m, 128)], ynA2[0:64],
                                 start=False, stop=True)
                nc.scalar.activation(hfm[:, m], ps, AF.Gelu_apprx_tanh,
                                     bias=sb["b1"][:, m:m + 1])
            for s in range(NSUB):
                ps = pp_late.tile([128, D], f32, tag="late", name="psf2")
                for m in range(6):
                    nc.tensor.matmul(ps, hfm[:, m, ts(s, 128)],
                                     sb["w2m"][:, m],
                                     start=(m == 0), stop=(m == 5))
                nc.vector.tensor_add(x_t[:, s, 0:D], x_t[:, s, 0:D], ps)
                if not biases_zero:
                    nc.vector.tensor_add(x_t[:, s, 0:D], x_t[:, s, 0:D],
                                         sb["b2"])

        for it in range(NTILES):
            x_t = xpool.tile([128, NSUB, DP], f32, tag="x", name="x_t")
            nc.sync.dma_start(
                out=x_t,
                in_=x_d[ts(it, TILE)].rearrange("(s p) f -> p s f", p=128))

            for li in range(DEPTH):
                sb = W[li]
                # ---- LN1 -> feature-major ----
                xnA, xnA2 = layernorm_to_fm(x_t, "1")
                # ---- q,k (feature-major out) ----
                qA = wpool.tile([128, TILE], bf16, tag="qA", name="qA")
                kA = wpool.tile([128, TILE], bf16, tag="kA", name="kA")
                qB = wpool.tile([64, TILE], bf16, tag="qB", name="qB")
                kB = wpool.tile([64, TILE], bf16, tag="kB", name="kB")
                for m in range(4):
                    mw = 128 if m < 2 else 64
                    moff = m * 128 if m < 2 else 256 + (m - 2) * 64
                    ps = pp_early.tile([mw, TILE], f32, tag="early", name="psqk")
                    nc.tensor.matmul(ps, sb["wqk0"][:, ds(moff, mw)], xnA,
                                     start=True, stop=False)
                    nc.tensor.matmul(ps, sb["wqk1"][:, ds(moff, mw)],
                                     xnA2[0:64], start=False, stop=True)
                    dst = (qA, kA, qB, kB)[m]
                    if biases_zero:
                        if m % 2 == 0:
                            nc.vector.tensor_copy(dst, ps)
                        else:
                            nc.scalar.activation(dst, ps, AF.Copy)
                    else:
                        nc.scalar.activation(dst, ps, AF.Identity,
                                             bias=sb["bqk"][0:mw, m:m + 1])
                # per-head base-0 copies (PE row tile position must be 0)
                q6 = wpool.tile([32, HEADS, TILE], bf16, tag="q6", name="q6")
                k6 = wpool.tile([32, HEADS, TILE], bf16, tag="k6", name="k6")
                for h in range(HEADS):
                    qsrc = qA[ts(h, 32)] if h < 4 else qB[ts(h - 4, 32)]
                    ksrc = kA[ts(h, 32)] if h < 4 else kB[ts(h - 4, 32)]
                    nc.gpsimd.dma_start(out=q6[:, h], in_=qsrc)
                    nc.gpsimd.dma_start(out=k6[:, h], in_=ksrc)
                # ---- v (token-major out) ----
                v_tm = wpool.tile([128, NSUB, D], bf16, tag="vtm", name="v_tm")
                for s in range(NSUB):
                    ps = pp_mid.tile([128, D], f32, tag="mid", name="psv")
                    nc.tensor.matmul(ps, xnA[:, ts(s, 128)], sb["wv0"],
                                     start=True, stop=False)
                    nc.tensor.matmul(ps, xnA2[0:64, ts(s, 128)], sb["wv1"],
                                     start=False, stop=True)
                    if s % 2 == 0:
                        nc.vector.tensor_copy(v_tm[:, s], ps)
                    else:
                        nc.scalar.activation(v_tm[:, s], ps, AF.Copy)
                # ---- attention: one cluster-pair (128 tokens) per sub ----
                ofmA = wpool.tile([128, TILE], bf16, tag="ofA", name="ofmA")
                ofmB = wpool.tile([64, TILE], bf16, tag="ofB", name="ofmB")
                for s in range(NSUB):
                    scA = pp_mid.tile([128, 3, 128], f32, tag="mid", name="scA")
                    scB = pp_mid.tile([128, 3, 128], f32, tag="mid", name="scB")
                    for h in range(HEADS):
                        sct = scA if h < 3 else scB
                        cols = ds(s * 128, 128)
                        nc.tensor.matmul(sct[:, h % 3], q6[:, h, cols],
                                         k6[:, h, cols],
                                         start=True, stop=True)
                    E = spool.tile([128, HEADS, 128], bf16, tag="E", name="E")
                    sums = stpool.tile([128, HEADS], f32, tag="sm", name="sums")
                    rsum = stpool.tile([128, HEADS], f32, tag="rs", name="rsum")
                    nc.scalar.activation(E[:, 0:3], scA, AF.Exp)
                    nc.scalar.activation(E[:, 3:6], scB, AF.Exp)
                    nc.vector.reduce_sum(sums[0:64], E[0:64, :, 0:64],
                                         axis=mybir.AxisListType.X)
                    nc.vector.reduce_sum(sums[64:128], E[64:128, :, 64:128],
                                         axis=mybir.AxisListType.X)
                    nc.vector.reciprocal(rsum, sums)
                    P = p_bufs[pair_ctr[0] % NPBUF]
                    pair_ctr[0] += 1
                    for half in range(2):
                        hs = ds(half * 64, 64)
                        rs_half = rsum[ds(half * 64, 64)]
                        rsum_b = bass.AP(tensor=rs_half.tensor,
                                         offset=rs_half.offset,
                                         ap=[*rs_half.ap, [0, 64]])
                        nc.gpsimd.tensor_tensor(
                            out=P[hs, :, hs], in0=E[hs, :, hs],
                            in1=rsum_b, op=OP.mult)
                    pT = pp_mid.tile([128, HEADS, 128], bf16, tag="mid", name="pT")
                    for h in range(HEADS):
                        nc.tensor.transpose(pT[:, h], P[:, h], ident)
                    pkm = spool.tile([128, HEADS, 128], bf16, tag="pkm",
                                     name="pkm")
                    if s % 2 == 0:
                        nc.vector.tensor_copy(pkm, pT)
                    else:
                        nc.scalar.activation(pkm, pT, AF.Copy)
                    oP = pp_mid.tile([128, 256], f32, tag="mid", name="oP")
                    for h in range(HEADS):
                        if h < 4:
                            out = oP[ts(h, 32), 0:128]
                            colpos = h * 32
                        else:
                            out = oP[ts(h - 4, 32), 128:256]
                            colpos = (h - 4) * 32
                        nc.tensor.matmul(out, v_tm[:, s, ts(h, 32)],
                                         pkm[:, h], start=True, stop=True,
                                         tile_position=(0, colpos))
                    if biases_zero:
                        nc.scalar.activation(ofmA[:, ts(s, 128)],
                                             oP[:, 0:128], AF.Copy)
                        nc.vector.tensor_copy(ofmB[:, ts(s, 128)],
                                              oP[0:64, 128:256])
                    else:
                        nc.scalar.activation(ofmA[:, ts(s, 128)],
                                             oP[:, 0:128], AF.Identity,
                                             bias=sb["bv"][:, 0:1])
                        nc.scalar.activation(ofmB[:, ts(s, 128)],
                                             oP[0:64, 128:256],
                                             AF.Identity,
                                             bias=sb["bv"][0:64, 1:2])
                # ---- proj + residual ----
                for s in range(NSUB):
                    ps = pp_late.tile([128, D], f32, tag="late", name="psp")
                    nc.tensor.matmul(ps, ofmA[:, ts(s, 128)], sb["wp0"],
                                     start=True, stop=False)
                    nc.tensor.matmul(ps, ofmB[:, ts(s, 128)], sb["wp1"],
                                     start=False, stop=True)
                    nc.vector.tensor_add(x_t[:, s, 0:D], x_t[:, s, 0:D], ps)
                    if not biases_zero:
                        nc.vector.tensor_add(x_t[:, s, 0:D], x_t[:, s, 0:D],
                                             sb["bp"])
                # ---- LN2 + MLP ----
                ynA, ynA2 = layernorm_to_fm(x_t, "2")
                mlp_block(sb, ynA, ynA2, x_t)

            nc.sync.dma_start(
                out=y_d[ts(it, TILE)].rearrange("(s p) f -> p s f", p=128),
                in_=x_t[:, :, 0:D])

    nc.compile()
    _COMPILED[key] = nc
    return nc


def _ensure_ntff_hook():
    """The image's antenv package lacks axon_hooks; synthesize it and install
    the ctypes-based NTFF profile hook from trn_agent_boot (test-only path)."""
    import sys, types
    if "antenv.axon_hooks" in sys.modules:
        return True
    try:
        mod = types.ModuleType("antenv.axon_hooks")
        state = {}
        mod.set_axon_ntff_profile_hook = lambda h: state.__setitem__("h", h)
        mod.get_axon_ntff_profile_hook = lambda: state.get("h")
        sys.modules["antenv.axon_hooks"] = mod
        import antenv
        antenv.axon_hooks = mod
        from trn_agent_boot.trn_boot import _ntff_profile_via_ctypes
        mod.set_axon_ntff_profile_hook(
            _ntff_profile_via_ctypes("/opt/axon/libaxon_pjrt.so"))
        return True
    except Exception as e:  # pragma: no cover
        print(f"NTFF hook shim failed: {e}")
        return False


def _run(inputs, trace=False):
    """Shard, execute on 8 cores, gather. Returns (y_full, exec_time_ns)."""
    from concourse.bass_utils import run_bass_kernel_spmd

    if trace:
        trace = _ensure_ntff_hook()

    layers = _fold_weights(inputs)
    bz = all(
        not np.any(np.asarray(d[k], np.float32))
        for d in layers for k in d if k.startswith(("bp", "b2")))
    nc = _build_nc(biases_zero=bz)

    x = np.asarray(inputs["x"], np.float32)
    pos = np.asarray(inputs["pos"], np.float32)
    w = int(np.asarray(inputs["w"]))
    order = _scanline_order(pos, w)
    x_ord = np.take_along_axis(x, order[..., None], axis=1)
    shards = np.zeros((NCORES, T, DP), np.float32)
    shards[:, :, 0:D] = x_ord.reshape(NCORES, T, D)

    wmap = {}
    for d in layers:
        wmap.update({k: np.ascontiguousarray(v) for k, v in d.items()})

    in_maps = [{"x": shards[c], **wmap} for c in range(NCORES)]
    res = run_bass_kernel_spmd(nc, in_maps, core_ids=list(range(NCORES)),
                               trace=trace)
    y_ord = np.stack([res.results[c]["y"] for c in range(NCORES)])
    y_ord = y_ord.reshape(B, N, D)
    y = np.empty_like(y_ord)
    np.put_along_axis(y, order[..., None], y_ord, axis=1)
    return y.astype(np.float32), res.exec_time_ns


def kernel(**inputs):
    y, _ = _run(inputs, trace=False)
    return y



# revision 15
# speedup vs baseline: 1.6541x; 1.6541x over previous
"""Trainium2 Bass kernel for nn_BasicLayer (sparse cluster attention, 2 layers).

v2 design
---------
Scanline order commutes with per-token ops, so gather to curve order on host,
run both layers on-device over contiguous 64-token clusters, scatter back.
8192 tokens/core (half a batch), weights replicated.

Per-core pipeline (TILE=1024 tokens = 8 subs of 128 = 16 cluster pairs):
- All layout flips (LN token-major -> feature-major, P -> P^T) go through the
  DMA crossbar (dma_start_transpose, batched), not the PE. No identity matmuls.
- q/k stored as 3-head groups [96, T] so per-head score matmuls slice at
  partition offsets {0,32,64} directly (offset 96 is unencodable).
- Scalar activation-table switches (Sqrt/Exp/Gelu) are prefetched with dummy
  ops so the ~1.5us ACT_TABLE_LOAD never sits on the critical path.
- LN chains (bn_stats/sqrt/recip/normalize/xbar) are emitted interleaved with
  the previous phase's residual adds, per 4-sub half, so the PE never waits
  on a cold LN chain at a phase boundary.
- Cross-cluster softmax leakage is killed by persistent zero off-diagonal
  blocks in P (normalize writes diagonal blocks only).
"""

import os
import numpy as np
import ml_dtypes

# ---- problem constants (hardcoded per contract) ----
B, N, D = 4, 16384, 192
HEADS, DH, CLM = 6, 32, 64
GRID_W = 128
DEPTH = 2
NCORES = 8
T = (B * N) // NCORES                # 8192 tokens per core
SUB = 128
NSUB = 8
TILE = SUB * NSUB                    # 1024-token supertile
NTILES = T // TILE                   # 8
DFF = 768

_STOP_PHASE = None   # debug: "qk"|"scores"|"attn"|None
_COMPILED = {}


def _scanline_order(pos, w):
    ix = np.floor(pos[..., 0]).astype(np.int64)
    iy = np.floor(pos[..., 1]).astype(np.int64)
    key = iy * w + np.where(iy % 2 == 1, w - 1 - ix, ix)
    return np.argsort(key, axis=1, kind="stable")


def _fold_weights(inputs):
    """Fold LN affine + biases into matmul weights. Returns per-layer dicts
    laid out exactly as the DRAM tensors the kernel declares."""
    bf16 = ml_dtypes.bfloat16
    scale = DH ** -0.5
    layers = []
    for i in range(DEPTH):
        g1 = np.asarray(inputs["ln1_g"][i], np.float64)
        b1 = np.asarray(inputs["ln1_b"][i], np.float64)
        Wqkv = np.asarray(inputs["w_qkv"][i], np.float64)
        bqkv = np.asarray(inputs["b_qkv"][i], np.float64)
        w_eff = g1[:, None] * Wqkv
        b_eff = b1 @ Wqkv + bqkv
        wq = w_eff[:, 0:D] * scale
        bq = b_eff[0:D] * scale
        wk = w_eff[:, D:2 * D]
        bk = b_eff[D:2 * D]
        wv = w_eff[:, 2 * D:3 * D]
        bv = b_eff[2 * D:3 * D]
        # qk weight M-layout: 3-head groups [q h0-2 | k h0-2 | q h3-5 | k h3-5]
        wqk = np.concatenate(
            [wq[:, :96], wk[:, :96], wq[:, 96:], wk[:, 96:]], axis=1)
        bqk = np.stack([bq[:96], bk[:96], bq[96:], bk[96:]], axis=1)
        wp = np.asarray(inputs["w_proj"][i], np.float64)
        bp = np.asarray(inputs["b_proj"][i], np.float64)
        g2 = np.asarray(inputs["ln2_g"][i], np.float64)
        b2 = np.asarray(inputs["ln2_b"][i], np.float64)
        W1 = np.asarray(inputs["w_fc1"][i], np.float64)
        w1_eff = g2[:, None] * W1
        b1_eff = b2 @ W1 + np.asarray(inputs["b_fc1"][i], np.float64)
        W2 = np.asarray(inputs["w_fc2"][i], np.float64)
        bfc2 = np.asarray(inputs["b_fc2"][i], np.float64)
        bv_t = np.stack([bv[:96], bv[96:]], axis=1)
        layers.append({
            f"wqk{i}": wqk.astype(bf16),
            f"bqk{i}": bqk.astype(np.float32),
            f"wv{i}": wv.astype(bf16),
            f"bv{i}": bv_t.astype(np.float32),
            f"wp{i}": wp.astype(bf16),
            f"bp{i}": np.tile(bp.astype(np.float32), (128, 1)),
            f"w1{i}": w1_eff.astype(bf16),
            f"b1{i}": b1_eff.reshape(6, 128).T.copy().astype(np.float32),
            f"w2{i}": W2.astype(bf16),
            f"b2{i}": np.tile(bfc2.astype(np.float32), (128, 1)),
        })
    return layers


def _build_nc(biases_zero=False, ntiles=NTILES):
    key = ("nc", biases_zero, ntiles)
    if key in _COMPILED:
        return _COMPILED[key]

    from contextlib import ExitStack
    import concourse.bass as bass
    import concourse.tile as tile
    from concourse import bacc, mybir
    from concourse.bass import ts, ds

    f32 = mybir.dt.float32
    bf16 = mybir.dt.bfloat16
    AF = mybir.ActivationFunctionType
    OP = mybir.AluOpType

    nc = bacc.Bacc("TRN2", target_bir_lowering=False, debug=False,
                   enable_asserts=False, num_devices=NCORES)

    x_d = nc.dram_tensor("x", [T, D], f32, kind="ExternalInput").ap()
    y_d = nc.dram_tensor("y", [T, D], f32, kind="ExternalOutput").ap()
    wd = []
    for i in range(DEPTH):
        wd.append({
            "wqk": nc.dram_tensor(f"wqk{i}", [D, 384], bf16, kind="ExternalInput").ap(),
            "bqk": nc.dram_tensor(f"bqk{i}", [96, 4], f32, kind="ExternalInput").ap(),
            "wv": nc.dram_tensor(f"wv{i}", [D, D], bf16, kind="ExternalInput").ap(),
            "bv": nc.dram_tensor(f"bv{i}", [96, 2], f32, kind="ExternalInput").ap(),
            "wp": nc.dram_tensor(f"wp{i}", [D, D], bf16, kind="ExternalInput").ap(),
            "bp": nc.dram_tensor(f"bp{i}", [128, D], f32, kind="ExternalInput").ap(),
            "w1": nc.dram_tensor(f"w1{i}", [D, DFF], bf16, kind="ExternalInput").ap(),
            "b1": nc.dram_tensor(f"b1{i}", [128, 6], f32, kind="ExternalInput").ap(),
            "w2": nc.dram_tensor(f"w2{i}", [DFF, D], bf16, kind="ExternalInput").ap(),
            "b2": nc.dram_tensor(f"b2{i}", [128, D], f32, kind="ExternalInput").ap(),
        })

    def bcast(ap2d, n):
        return bass.AP(tensor=ap2d.tensor, offset=ap2d.offset,
                       ap=[*ap2d.ap, [0, n]])

    with tile.TileContext(nc) as tc, ExitStack() as ctx:
        consts = ctx.enter_context(tc.tile_pool(name="consts", bufs=1))
        xpool = ctx.enter_context(tc.tile_pool(name="xpool", bufs=3))
        lnpool = ctx.enter_context(tc.tile_pool(name="lnpool", bufs=3))
        qkpool = ctx.enter_context(tc.tile_pool(name="qkpool", bufs=2))
        apool = ctx.enter_context(tc.tile_pool(name="apool", bufs=3))
        mpool = ctx.enter_context(tc.tile_pool(name="mpool", bufs=2))
        stpool = ctx.enter_context(tc.tile_pool(name="stpool", bufs=8))
        pp_sc = ctx.enter_context(tc.tile_pool(name="pp_sc", bufs=4, space="PSUM"))
        pp_h = ctx.enter_context(tc.tile_pool(name="pp_h", bufs=2, space="PSUM"))
        pp_sm = ctx.enter_context(tc.tile_pool(name="pp_sm", bufs=2, space="PSUM"))

        eps_t = consts.tile([128, 1], f32)
        nc.vector.memset(eps_t, 1e-5)
        dsrc = consts.tile([128, 1], f32)
        nc.vector.memset(dsrc, 0.5)
        ddst = consts.tile([128, 1], f32, name="ddst")

        def prefetch(af):
            nc.scalar.activation(ddst, dsrc, af)

        # persistent softmax tiles: off-diagonal (cross-cluster) blocks stay 0
        NPBUF = 4
        p_bufs = []
        for pb_i in range(NPBUF):
            pb = consts.tile([128, HEADS, 128], bf16, name=f"pbuf{pb_i}")
            nc.vector.memset(pb, 0.0)
            p_bufs.append(pb)
        pctr = [0]

        # --- weights to SBUF once ---
        W = []
        for i in range(DEPTH):
            d = wd[i]
            sb = {}
            sb["wqkA"] = consts.tile([128, 384], bf16, name=f"wqkA{i}")
            sb["wqkB"] = consts.tile([128, 384], bf16, name=f"wqkB{i}")
            nc.sync.dma_start(out=sb["wqkA"], in_=d["wqk"][0:128])
            nc.sync.dma_start(out=sb["wqkB"][ds(64, 64)], in_=d["wqk"][128:192])
            sb["wvA"] = consts.tile([128, D], bf16, name=f"wvA{i}")
            sb["wvB"] = consts.tile([128, D], bf16, name=f"wvB{i}")
            nc.sync.dma_start(out=sb["wvA"], in_=d["wv"][0:128])
            nc.sync.dma_start(out=sb["wvB"][ds(64, 64)], in_=d["wv"][128:192])
            sb["wpA"] = consts.tile([96, D], bf16, name=f"wpA{i}")
            sb["wpB"] = consts.tile([96, D], bf16, name=f"wpB{i}")
            nc.sync.dma_start(out=sb["wpA"], in_=d["wp"][0:96])
            nc.sync.dma_start(out=sb["wpB"], in_=d["wp"][96:192])
            sb["w1A"] = consts.tile([128, DFF], bf16, name=f"w1A{i}")
            sb["w1B"] = consts.tile([128, DFF], bf16, name=f"w1B{i}")
            nc.sync.dma_start(out=sb["w1A"], in_=d["w1"][0:128])
            nc.sync.dma_start(out=sb["w1B"][ds(64, 64)], in_=d["w1"][128:192])
            sb["w2m"] = consts.tile([128, 6, D], bf16, name=f"w2m{i}")
            nc.sync.dma_start(
                out=sb["w2m"],
                in_=d["w2"].rearrange("(m p) n -> p m n", p=128))
            for nm, shp in (("bqk", [96, 4]), ("bv", [96, 2]), ("b1", [128, 6]),
                            ("bp", [128, D]), ("b2", [128, D])):
                sb[nm] = consts.tile(shp, f32, name=f"{nm}{i}")
                nc.sync.dma_start(out=sb[nm], in_=d[nm])
            W.append(sb)

        # ---------- LN chain helpers ----------
        def ln_tiles():
            return {
                "mv": lnpool.tile([128, NSUB, 2], f32, tag="mv", name="mv"),
                "sd": lnpool.tile([128, NSUB], f32, tag="sd", name="sd"),
                "r": lnpool.tile([128, NSUB], f32, tag="r", name="r"),
                "xnA": lnpool.tile([128, NSUB, 128], bf16, tag="xnA", name="xnA"),
                "xnB": lnpool.tile([128, NSUB, 128], bf16, tag="xnB", name="xnB"),
                "fmA": lnpool.tile([128, NSUB, 128], bf16, tag="fmA", name="fmA"),
                "fmB": lnpool.tile([128, NSUB, 128], bf16, tag="fmB", name="fmB"),
            }

        def ln_stats(ln, x_t, s):
            st = stpool.tile([128, 6], f32, tag="st", name="st")
            nc.vector.bn_stats(st, x_t[:, s, 0:D])
            nc.vector.bn_aggr(ln["mv"][:, s], st)

        def ln_finish_half(ln, x_t, h):
            s0 = 4 * h
            nc.scalar.activation(ln["sd"][:, s0:s0 + 4],
                                 ln["mv"][:, s0:s0 + 4, 1], AF.Sqrt,
                                 bias=eps_t)
            nc.vector.reciprocal(ln["r"][:, s0:s0 + 4], ln["sd"][:, s0:s0 + 4])
            for s in range(s0, s0 + 4):
                mu = ln["mv"][:, s, 0:1]
                rr = ln["r"][:, s:s + 1]
                nc.gpsimd.tensor_scalar(
                    out=ln["xnA"][:, s], in0=x_t[:, s, 0:128],
                    scalar1=mu, scalar2=rr, op0=OP.subtract, op1=OP.mult)
                nc.vector.tensor_scalar(
                    out=ln["xnB"][:, s], in0=x_t[:, s, 64:192],
                    scalar1=mu, scalar2=rr, op0=OP.subtract, op1=OP.mult)
            nc.sync.dma_start_transpose(out=ln["fmA"][:, s0:s0 + 4, :],
                                        in_=ln["xnA"][:, s0:s0 + 4, :])
            nc.sync.dma_start_transpose(out=ln["fmB"][:, s0:s0 + 4, :],
                                        in_=ln["xnB"][:, s0:s0 + 4, :])

        def fmA_half(ln, h):
            return ln["fmA"][:, 4 * h:4 * h + 4, :].rearrange("p a b -> p (a b)")

        def fmB_half(ln, h):
            return ln["fmB"][ds(64, 64), 4 * h:4 * h + 4, :].rearrange(
                "p a b -> p (a b)")

        # ======================================================================
        def emit_layer(sb, x_t, ln1, x_nxt, nxt_is_tile):
            """Emit one transformer layer. ln1 holds this layer's LN1 (already
            emitted). Returns the LN tiles for the NEXT layer's LN1 (computed
            on x_nxt: either this x_t post-MLP, or the next tile's x)."""
            # ---- qk: 4 blocks x 2 halves ----
            qA = qkpool.tile([96, TILE], bf16, tag="qA", name="qA")
            kA = qkpool.tile([96, TILE], bf16, tag="kA", name="kA")
            qB = qkpool.tile([96, TILE], bf16, tag="qB", name="qB")
            kB = qkpool.tile([96, TILE], bf16, tag="kB", name="kB")
            dsts = (qA, kA, qB, kB)
            for hf in range(2):
                rA, rB = fmA_half(ln1, hf), fmB_half(ln1, hf)
                for m in range(4):
                    ps = pp_h.tile([96, 512], f32, tag="ph", name="psqk")
                    nc.tensor.matmul(ps, sb["wqkA"][:, ds(96 * m, 96)], rA,
                                     start=True, stop=False)
                    nc.tensor.matmul(ps, sb["wqkB"][ds(64, 64), ds(96 * m, 96)], rB,
                                     start=False, stop=True)
                    dst = dsts[m][:, ts(hf, 512)]
                    if not biases_zero:
                        nc.scalar.activation(dst, ps, AF.Identity,
                                             bias=sb["bqk"][:, m:m + 1])
                    elif m % 2 == 0:
                        nc.vector.tensor_copy(dst, ps)
                    else:
                        nc.scalar.activation(dst, ps, AF.Copy)

            if _STOP_PHASE == "qk":
                return ln1
            # ---- scores + softmax + v, per sub ----
            v_tm = apool.tile([128, NSUB, D], bf16, tag="vtm", name="v_tm")
            pkms = []
            for s in range(NSUB):
                # one PSUM tile per PE tile-row: heads (hh, hh+3) share row
                # 32*hh; independent accumulation groups at different rows in
                # one PSUM tile lock up the device
                scR = [pp_sc.tile([128, 2, 128], f32, tag="sc", name=f"scR{r}")
                       for r in range(3)]
                for hd in range(HEADS):
                    grp, hh = hd // 3, hd % 3
                    qsrc = (qA, qB)[grp]
                    ksrc = (kA, kB)[grp]
                    nc.tensor.matmul(scR[hh][:, grp],
                                     qsrc[ds(32 * hh, 32), ts(s, 128)],
                                     ksrc[ds(32 * hh, 32), ts(s, 128)],
                                     start=True, stop=True)
                if _STOP_PHASE == "sconly":
                    continue
                # v for this sub (PE filler between score groups)
                psv = pp_sm.tile([128, D], f32, tag="sm", name="psv")
                nc.tensor.matmul(psv, ln1["fmA"][:, s, :], sb["wvA"],
                                 start=True, stop=False)
                nc.tensor.matmul(psv, ln1["fmB"][ds(64, 64), s, :], sb["wvB"][ds(64, 64)],
                                 start=False, stop=True)
                if s % 2 == 0:
                    nc.scalar.activation(v_tm[:, s], psv, AF.Copy)
                else:
                    nc.vector.tensor_copy(v_tm[:, s], psv)
                if _STOP_PHASE == "sv":
                    continue
                # softmax (diagonal 64-blocks only)
                E = apool.tile([128, HEADS, 128], bf16, tag="E", name="E")
                sums = stpool.tile([128, HEADS], f32, tag="sm", name="sums")
                rsum = stpool.tile([128, HEADS], f32, tag="rs", name="rsum")
                for hh in range(3):
                    for half in range(2):
                        ho = half * 64
                        ebase = E[ds(ho, 64)]
                        esl = bass.AP(
                            tensor=ebase.tensor,
                            offset=ebase.offset + hh * 128 + ho,
                            ap=[ebase.ap[0], [3 * 128, 2], [1, 64]])
                        nc.scalar.activation(
                            esl, scR[hh][ds(ho, 64), :, ho:ho + 64], AF.Exp)
                if _STOP_PHASE == "exp":
                    continue
                nc.vector.reduce_sum(sums[0:64], E[0:64, :, 0:64],
                                     axis=mybir.AxisListType.X)
                nc.vector.reduce_sum(sums[ds(64, 64)], E[ds(64, 64), :, 64:128],
                                     axis=mybir.AxisListType.X)
                nc.vector.reciprocal(rsum, sums)
                if _STOP_PHASE == "sums":
                    continue
                P = p_bufs[pctr[0] % NPBUF]
                pctr[0] += 1
                for half in range(2):
                    hs = ds(half * 64, 64)
                    rs_half = rsum[ds(half * 64, 64)]
                    nc.gpsimd.tensor_tensor(
                        out=P[hs, :, hs], in0=E[hs, :, hs],
                        in1=bcast(rs_half, 64), op=OP.mult)
                if _STOP_PHASE == "norm":
                    continue
                pkm = apool.tile([128, HEADS, 128], bf16, tag="pkm", name="pkm")
                nc.sync.dma_start_transpose(
                    out=pkm, in_=P.rearrange("p a b -> p (a b)"))
                pkms.append(pkm)

            if _STOP_PHASE in ("sconly", "sconly2", "sv", "exp", "sums", "norm", "scores"):
                return ln1
            if biases_zero:
                prefetch(AF.Sqrt)

            # ---- O + out-copies + proj + residual + LN2 stats, per sub ----
            ln2 = ln_tiles()
            ofm0 = apool.tile([96, TILE], bf16, tag="ofm0", name="ofm0")
            ofm1 = apool.tile([96, TILE], bf16, tag="ofm1", name="ofm1")
            for s in range(NSUB):
                oP = pp_sm.tile([96, 256], f32, tag="sm", name="oP")
                for hd in range(HEADS):
                    grp, hh = hd // 3, hd % 3
                    nc.tensor.matmul(oP[ds(32 * hh, 32), ts(grp, 128)],
                                     v_tm[:, s, ts(hd, 32)], pkms[s][:, hd],
                                     start=True, stop=True,
                                     tile_position=(0, 32 * hh))
                if not biases_zero:
                    nc.scalar.activation(ofm0[:, ts(s, 128)], oP[:, 0:128],
                                         AF.Identity, bias=sb["bv"][:, 0:1])
                    nc.scalar.activation(ofm1[:, ts(s, 128)], oP[:, 128:256],
                                         AF.Identity, bias=sb["bv"][:, 1:2])
                elif s % 2 == 0:
                    nc.vector.tensor_copy(ofm0[:, ts(s, 128)], oP[:, 0:128])
                    nc.scalar.activation(ofm1[:, ts(s, 128)], oP[:, 128:256],
                                         AF.Copy)
                else:
                    nc.scalar.activation(ofm0[:, ts(s, 128)], oP[:, 0:128],
                                         AF.Copy)
                    nc.vector.tensor_copy(ofm1[:, ts(s, 128)], oP[:, 128:256])
                psp = pp_sm.tile([128, D], f32, tag="sm", name="psp")
                nc.tensor.matmul(psp, ofm0[:, ts(s, 128)], sb["wpA"],
                                 start=True, stop=False)
                nc.tensor.matmul(psp, ofm1[:, ts(s, 128)], sb["wpB"],
                                 start=False, stop=True)
                nc.vector.tensor_add(x_t[:, s, 0:D], x_t[:, s, 0:D], psp)
                if not biases_zero:
                    nc.vector.tensor_add(x_t[:, s, 0:D], x_t[:, s, 0:D],
                                         sb["bp"])
                ln_stats(ln2, x_t, s)
                if s == 3:
                    if not biases_zero:
                        prefetch(AF.Sqrt)
                    ln_finish_half(ln2, x_t, 0)
                elif s == 7:
                    ln_finish_half(ln2, x_t, 1)
                    prefetch(AF.Gelu_apprx_tanh)

            if _STOP_PHASE == "attn":
                return ln2
            # ---- MLP: fc1+gelu both halves, then fc2 + residual ----
            hfms = []
            for hf in range(2):
                rA, rB = fmA_half(ln2, hf), fmB_half(ln2, hf)
                hfm = mpool.tile([128, 6, 512], bf16, tag="hfm", name="hfm")
                for m in range(6):
                    ps = pp_h.tile([128, 512], f32, tag="ph", name="psh")
                    nc.tensor.matmul(ps, sb["w1A"][:, ts(m, 128)], rA,
                                     start=True, stop=False)
                    nc.tensor.matmul(ps, sb["w1B"][ds(64, 64), ts(m, 128)], rB,
                                     start=False, stop=True)
                    if biases_zero:
                        nc.scalar.activation(hfm[:, m], ps, AF.Gelu_apprx_tanh)
                    else:
                        nc.scalar.activation(hfm[:, m], ps, AF.Gelu_apprx_tanh,
                                             bias=sb["b1"][:, m:m + 1])
                hfms.append(hfm)
            ln_n = ln_tiles()
            for hf in range(2):
                for si in range(4):
                    s = 4 * hf + si
                    ps = pp_sm.tile([128, D], f32, tag="sm", name="psf2")
                    for m in range(6):
                        nc.tensor.matmul(ps, hfms[hf][:, m, ts(si, 128)],
                                         sb["w2m"][:, m],
                                         start=(m == 0), stop=(m == 5))
                    nc.vector.tensor_add(x_t[:, s, 0:D], x_t[:, s, 0:D], ps)
                    if not biases_zero:
                        nc.vector.tensor_add(x_t[:, s, 0:D], x_t[:, s, 0:D],
                                             sb["b2"])
                    if x_nxt is None:
                        continue
                    if not nxt_is_tile:
                        ln_stats(ln_n, x_t, s)
                if x_nxt is None:
                    continue
                if not nxt_is_tile:
                    if hf == 0:
                        prefetch(AF.Sqrt)
                        ln_finish_half(ln_n, x_t, 0)
                    else:
                        ln_finish_half(ln_n, x_t, 1)
                        prefetch(AF.Exp)
                else:
                    # next LN is on the next tile's x (independent data)
                    if hf == 0:
                        prefetch(AF.Sqrt)
                        for s2 in range(4):
                            ln_stats(ln_n, x_nxt, s2)
                        ln_finish_half(ln_n, x_nxt, 0)
                    else:
                        for s2 in range(4, 8):
                            ln_stats(ln_n, x_nxt, s2)
                        ln_finish_half(ln_n, x_nxt, 1)
                        prefetch(AF.Exp)
            return ln_n

        # ======================================================================
        def load_tile(it):
            xt = xpool.tile([128, NSUB, D], f32, tag="x", name="x_t")
            nc.sync.dma_start(
                out=xt, in_=x_d[ts(it, TILE)].rearrange("(s p) f -> p s f",
                                                        p=128))
            return xt

        x_cur = load_tile(0)
        ln = ln_tiles()
        prefetch(AF.Sqrt)
        for s in range(4):
            ln_stats(ln, x_cur, s)
        ln_finish_half(ln, x_cur, 0)
        for s in range(4, 8):
            ln_stats(ln, x_cur, s)
        ln_finish_half(ln, x_cur, 1)
        prefetch(AF.Exp)

        for it in range(ntiles):
            x_nxt = load_tile(it + 1) if it + 1 < ntiles else None
            # layer 0: next LN is layer 1's LN1 on this x_t
            ln = emit_layer(W[0], x_cur, ln, x_cur, nxt_is_tile=False)
            # layer 1: next LN is the next tile's LN1 on x_nxt
            ln = emit_layer(W[1], x_cur, ln, x_nxt, nxt_is_tile=True)
            nc.sync.dma_start(
                out=y_d[ts(it, TILE)].rearrange("(s p) f -> p s f", p=128),
                in_=x_cur[:, :, 0:D])
            x_cur = x_nxt

    nc.compile()
    _COMPILED[key] = nc
    return nc


def _ensure_ntff_hook():
    """The image's antenv package lacks axon_hooks; synthesize it and install
    the ctypes-based NTFF profile hook from trn_agent_boot (test-only path)."""
    import sys, types
    if "antenv.axon_hooks" in sys.modules:
        return True
    try:
        mod = types.ModuleType("antenv.axon_hooks")
        state = {}
        mod.set_axon_ntff_profile_hook = lambda h: state.__setitem__("h", h)
        mod.get_axon_ntff_profile_hook = lambda: state.get("h")
        sys.modules["antenv.axon_hooks"] = mod
        import antenv
        antenv.axon_hooks = mod
        from trn_agent_boot.trn_boot import _ntff_profile_via_ctypes
        mod.set_axon_ntff_profile_hook(
            _ntff_profile_via_ctypes("/opt/axon/libaxon_pjrt.so"))
        return True
    except Exception as e:  # pragma: no cover
        print(f"NTFF hook shim failed: {e}")
        return False


def _run(inputs, trace=False):
    """Shard, execute on 8 cores, gather. Returns (y_full, exec_time_ns)."""
    from concourse.bass_utils import run_bass_kernel_spmd

    if trace:
        trace = _ensure_ntff_hook()

    layers = _fold_weights(inputs)
    bz = all(
        not np.any(np.asarray(d[k], np.float32))
        for d in layers for k in d
        if k.startswith(("bp", "b2", "bqk", "bv", "b1")))
    nc = _build_nc(biases_zero=bz)

    x = np.asarray(inputs["x"], np.float32)
    pos = np.asarray(inputs["pos"], np.float32)
    w = int(np.asarray(inputs["w"]))
    order = _scanline_order(pos, w)
    x_ord = np.take_along_axis(x, order[..., None], axis=1)
    shards = np.ascontiguousarray(x_ord.reshape(NCORES, T, D))

    wmap = {}
    for d in layers:
        wmap.update({k: np.ascontiguousarray(v) for k, v in d.items()})

    in_maps = [{"x": shards[c], **wmap} for c in range(NCORES)]
    res = run_bass_kernel_spmd(nc, in_maps, core_ids=list(range(NCORES)),
                               trace=trace)
    y_ord = np.stack([res.results[c]["y"] for c in range(NCORES)])
    y_ord = y_ord.reshape(B, N, D)
    y = np.empty_like(y_ord)
    np.put_along_axis(y, order[..., None], y_ord, axis=1)
    return y.astype(np.float32), res.exec_time_ns


def kernel(**inputs):
    y, _ = _run(inputs, trace=False)
    return y


# revision 17
# speedup vs baseline: 2.1615x; 1.3067x over previous
"""Trainium2 Bass kernel for nn_BasicLayer (sparse cluster attention, 2 layers).

v2 design
---------
Scanline order commutes with per-token ops, so gather to curve order on host,
run both layers on-device over contiguous 64-token clusters, scatter back.
8192 tokens/core (half a batch), weights replicated.

Per-core pipeline (TILE=1024 tokens = 8 subs of 128 = 16 cluster pairs):
- All layout flips (LN token-major -> feature-major, P -> P^T) go through the
  DMA crossbar (dma_start_transpose, batched), not the PE. No identity matmuls.
- q/k stored as 3-head groups [96, T] so per-head score matmuls slice at
  partition offsets {0,32,64} directly (offset 96 is unencodable).
- Scalar activation-table switches (Sqrt/Exp/Gelu) are prefetched with dummy
  ops so the ~1.5us ACT_TABLE_LOAD never sits on the critical path.
- LN chains (bn_stats/sqrt/recip/normalize/xbar) are emitted interleaved with
  the previous phase's residual adds, per 4-sub half, so the PE never waits
  on a cold LN chain at a phase boundary.
- Cross-cluster softmax leakage is killed by persistent zero off-diagonal
  blocks in P (normalize writes diagonal blocks only).
"""

import os
import numpy as np
import ml_dtypes

# ---- problem constants (hardcoded per contract) ----
B, N, D = 4, 16384, 192
HEADS, DH, CLM = 6, 32, 64
GRID_W = 128
DEPTH = 2
NCORES = 8
T = (B * N) // NCORES                # 8192 tokens per core
SUB = 128
NSUB = 8
TILE = SUB * NSUB                    # 1024-token supertile
NTILES = T // TILE                   # 8
DFF = 768

_STOP_PHASE = None   # debug: "qk"|"scores"|"attn"|None
_COMPILED = {}


def _scanline_order(pos, w):
    ix = np.floor(pos[..., 0]).astype(np.int64)
    iy = np.floor(pos[..., 1]).astype(np.int64)
    key = iy * w + np.where(iy % 2 == 1, w - 1 - ix, ix)
    return np.argsort(key, axis=1, kind="stable")


def _fold_weights(inputs):
    """Fold LN affine + biases into matmul weights. Returns per-layer dicts
    laid out exactly as the DRAM tensors the kernel declares."""
    bf16 = ml_dtypes.bfloat16
    scale = DH ** -0.5
    layers = []
    for i in range(DEPTH):
        g1 = np.asarray(inputs["ln1_g"][i], np.float64)
        b1 = np.asarray(inputs["ln1_b"][i], np.float64)
        Wqkv = np.asarray(inputs["w_qkv"][i], np.float64)
        bqkv = np.asarray(inputs["b_qkv"][i], np.float64)
        w_eff = g1[:, None] * Wqkv
        b_eff = b1 @ Wqkv + bqkv
        wq = w_eff[:, 0:D] * scale
        bq = b_eff[0:D] * scale
        wk = w_eff[:, D:2 * D]
        bk = b_eff[D:2 * D]
        wv = w_eff[:, 2 * D:3 * D]
        bv = b_eff[2 * D:3 * D]
        # qk weight M-layout: 3-head groups [q h0-2 | k h0-2 | q h3-5 | k h3-5]
        wqk = np.concatenate(
            [wq[:, :96], wk[:, :96], wq[:, 96:], wk[:, 96:]], axis=1)
        bqk = np.stack([bq[:96], bk[:96], bq[96:], bk[96:]], axis=1)
        wp = np.asarray(inputs["w_proj"][i], np.float64)
        bp = np.asarray(inputs["b_proj"][i], np.float64)
        g2 = np.asarray(inputs["ln2_g"][i], np.float64)
        b2 = np.asarray(inputs["ln2_b"][i], np.float64)
        W1 = np.asarray(inputs["w_fc1"][i], np.float64)
        w1_eff = g2[:, None] * W1
        b1_eff = b2 @ W1 + np.asarray(inputs["b_fc1"][i], np.float64)
        W2 = np.asarray(inputs["w_fc2"][i], np.float64)
        bfc2 = np.asarray(inputs["b_fc2"][i], np.float64)
        bv_t = np.stack([bv[:96], bv[96:]], axis=1)
        layers.append({
            f"wqk{i}": wqk.astype(bf16),
            f"bqk{i}": bqk.astype(np.float32),
            f"wv{i}": wv.astype(bf16),
            f"bv{i}": bv_t.astype(np.float32),
            f"wp{i}": wp.astype(bf16),
            f"bp{i}": np.tile(bp.astype(np.float32), (128, 1)),
            f"w1{i}": w1_eff.astype(bf16),
            f"b1{i}": b1_eff.reshape(6, 128).T.copy().astype(np.float32),
            f"w2{i}": W2.astype(bf16),
            f"b2{i}": np.tile(bfc2.astype(np.float32), (128, 1)),
        })
    return layers


def _build_nc(biases_zero=False, ntiles=NTILES):
    key = ("nc", biases_zero, ntiles)
    if key in _COMPILED:
        return _COMPILED[key]

    from contextlib import ExitStack
    import concourse.bass as bass
    import concourse.tile as tile
    from concourse import bacc, mybir
    from concourse.bass import ts, ds

    f32 = mybir.dt.float32
    bf16 = mybir.dt.bfloat16
    AF = mybir.ActivationFunctionType
    OP = mybir.AluOpType

    nc = bacc.Bacc("TRN2", target_bir_lowering=False, debug=False,
                   enable_asserts=False, num_devices=NCORES)

    x_d = nc.dram_tensor("x", [T, D], f32, kind="ExternalInput").ap()
    y_d = nc.dram_tensor("y", [T, D], f32, kind="ExternalOutput").ap()
    wd = []
    for i in range(DEPTH):
        wd.append({
            "wqk": nc.dram_tensor(f"wqk{i}", [D, 384], bf16, kind="ExternalInput").ap(),
            "bqk": nc.dram_tensor(f"bqk{i}", [96, 4], f32, kind="ExternalInput").ap(),
            "wv": nc.dram_tensor(f"wv{i}", [D, D], bf16, kind="ExternalInput").ap(),
            "bv": nc.dram_tensor(f"bv{i}", [96, 2], f32, kind="ExternalInput").ap(),
            "wp": nc.dram_tensor(f"wp{i}", [D, D], bf16, kind="ExternalInput").ap(),
            "bp": nc.dram_tensor(f"bp{i}", [128, D], f32, kind="ExternalInput").ap(),
            "w1": nc.dram_tensor(f"w1{i}", [D, DFF], bf16, kind="ExternalInput").ap(),
            "b1": nc.dram_tensor(f"b1{i}", [128, 6], f32, kind="ExternalInput").ap(),
            "w2": nc.dram_tensor(f"w2{i}", [DFF, D], bf16, kind="ExternalInput").ap(),
            "b2": nc.dram_tensor(f"b2{i}", [128, D], f32, kind="ExternalInput").ap(),
        })

    def bcast(ap2d, n):
        return bass.AP(tensor=ap2d.tensor, offset=ap2d.offset,
                       ap=[*ap2d.ap, [0, n]])

    with tile.TileContext(nc) as tc, ExitStack() as ctx:
        consts = ctx.enter_context(tc.tile_pool(name="consts", bufs=1))
        xpool = ctx.enter_context(tc.tile_pool(name="xpool", bufs=3))
        lnpool = ctx.enter_context(tc.tile_pool(name="lnpool", bufs=3))
        qkpool = ctx.enter_context(tc.tile_pool(name="qkpool", bufs=2))
        apool = ctx.enter_context(tc.tile_pool(name="apool", bufs=3))
        mpool = ctx.enter_context(tc.tile_pool(name="mpool", bufs=2))
        stpool = ctx.enter_context(tc.tile_pool(name="stpool", bufs=8))
        pp_sc = ctx.enter_context(tc.tile_pool(name="pp_sc", bufs=4, space="PSUM"))
        pp_h = ctx.enter_context(tc.tile_pool(name="pp_h", bufs=2, space="PSUM"))
        pp_sm = ctx.enter_context(tc.tile_pool(name="pp_sm", bufs=2, space="PSUM"))

        eps_t = consts.tile([128, 1], f32)
        nc.vector.memset(eps_t, 1e-5)
        dsrc = consts.tile([128, 1], f32)
        nc.vector.memset(dsrc, 0.5)
        ddst = consts.tile([128, 1], f32, name="ddst")

        def prefetch(af):
            nc.scalar.activation(ddst, dsrc, af)

        # persistent softmax tiles: off-diagonal (cross-cluster) blocks stay 0
        NPBUF = 4
        p_bufs = []
        for pb_i in range(NPBUF):
            pb = consts.tile([128, HEADS, 128], bf16, name=f"pbuf{pb_i}")
            nc.vector.memset(pb, 0.0)
            p_bufs.append(pb)
        pctr = [0]

        # --- weights to SBUF once ---
        W = []
        for i in range(DEPTH):
            d = wd[i]
            sb = {}
            sb["wqkA"] = consts.tile([128, 384], bf16, name=f"wqkA{i}")
            sb["wqkB"] = consts.tile([128, 384], bf16, name=f"wqkB{i}")
            nc.sync.dma_start(out=sb["wqkA"], in_=d["wqk"][0:128])
            nc.sync.dma_start(out=sb["wqkB"][ds(64, 64)], in_=d["wqk"][128:192])
            sb["wvA"] = consts.tile([128, D], bf16, name=f"wvA{i}")
            sb["wvB"] = consts.tile([128, D], bf16, name=f"wvB{i}")
            nc.sync.dma_start(out=sb["wvA"], in_=d["wv"][0:128])
            nc.sync.dma_start(out=sb["wvB"][ds(64, 64)], in_=d["wv"][128:192])
            sb["wpA"] = consts.tile([96, D], bf16, name=f"wpA{i}")
            sb["wpB"] = consts.tile([96, D], bf16, name=f"wpB{i}")
            nc.sync.dma_start(out=sb["wpA"], in_=d["wp"][0:96])
            nc.sync.dma_start(out=sb["wpB"], in_=d["wp"][96:192])
            sb["w1A"] = consts.tile([128, DFF], bf16, name=f"w1A{i}")
            sb["w1B"] = consts.tile([128, DFF], bf16, name=f"w1B{i}")
            nc.sync.dma_start(out=sb["w1A"], in_=d["w1"][0:128])
            nc.sync.dma_start(out=sb["w1B"][ds(64, 64)], in_=d["w1"][128:192])
            sb["w2m"] = consts.tile([128, 6, D], bf16, name=f"w2m{i}")
            nc.sync.dma_start(
                out=sb["w2m"],
                in_=d["w2"].rearrange("(m p) n -> p m n", p=128))
            for nm, shp in (("bqk", [96, 4]), ("bv", [96, 2]), ("b1", [128, 6]),
                            ("bp", [128, D]), ("b2", [128, D])):
                sb[nm] = consts.tile(shp, f32, name=f"{nm}{i}")
                nc.sync.dma_start(out=sb[nm], in_=d[nm])
            W.append(sb)

        # ---------- LN chain helpers ----------
        def ln_tiles():
            return {
                "mv": lnpool.tile([128, NSUB, 2], f32, tag="mv", name="mv"),
                "sd": lnpool.tile([128, NSUB], f32, tag="sd", name="sd"),
                "r": lnpool.tile([128, NSUB], f32, tag="r", name="r"),
                "xnA": lnpool.tile([128, NSUB, 128], bf16, tag="xnA", name="xnA"),
                "xnB": lnpool.tile([128, NSUB, 128], bf16, tag="xnB", name="xnB"),
                "fmA": lnpool.tile([128, NSUB, 128], bf16, tag="fmA", name="fmA"),
                "fmB": lnpool.tile([128, NSUB, 128], bf16, tag="fmB", name="fmB"),
            }

        def ln_stats(ln, x_t, s):
            st = stpool.tile([128, 6], f32, tag="st", name="st")
            nc.vector.bn_stats(st, x_t[:, s, 0:D])
            nc.vector.bn_aggr(ln["mv"][:, s], st)

        def ln_finish_half(ln, x_t, h):
            s0 = 4 * h
            nc.scalar.activation(ln["sd"][:, s0:s0 + 4],
                                 ln["mv"][:, s0:s0 + 4, 1], AF.Sqrt,
                                 bias=eps_t)
            nc.vector.reciprocal(ln["r"][:, s0:s0 + 4], ln["sd"][:, s0:s0 + 4])
            for s in range(s0, s0 + 4):
                mu = ln["mv"][:, s, 0:1]
                rr = ln["r"][:, s:s + 1]
                mu_b = bcast(mu, 128)
                r_b = bcast(rr, 128)
                tA = stpool.tile([128, 128], bf16, tag="tA", name="tA")
                nc.gpsimd.tensor_tensor(out=tA, in0=x_t[:, s, 0:128],
                                        in1=mu_b, op=OP.subtract)
                nc.gpsimd.tensor_tensor(out=ln["xnA"][:, s], in0=tA,
                                        in1=r_b, op=OP.mult)
                tB = stpool.tile([128, 128], bf16, tag="tB", name="tB")
                eng = nc.vector if s % 2 == 0 else nc.gpsimd
                eng.tensor_tensor(out=tB, in0=x_t[:, s, 64:192],
                                  in1=mu_b, op=OP.subtract)
                eng.tensor_tensor(out=ln["xnB"][:, s], in0=tB,
                                  in1=r_b, op=OP.mult)
            nc.sync.dma_start_transpose(out=ln["fmA"][:, s0:s0 + 4, :],
                                        in_=ln["xnA"][:, s0:s0 + 4, :])
            nc.sync.dma_start_transpose(out=ln["fmB"][:, s0:s0 + 4, :],
                                        in_=ln["xnB"][:, s0:s0 + 4, :])

        def fmA_half(ln, h):
            return ln["fmA"][:, 4 * h:4 * h + 4, :].rearrange("p a b -> p (a b)")

        def fmB_half(ln, h):
            return ln["fmB"][ds(64, 64), 4 * h:4 * h + 4, :].rearrange(
                "p a b -> p (a b)")

        # ======================================================================
        def emit_layer(sb, x_t, ln1, x_nxt, nxt_is_tile):
            """Emit one transformer layer. ln1 holds this layer's LN1 (already
            emitted). Returns the LN tiles for the NEXT layer's LN1 (computed
            on x_nxt: either this x_t post-MLP, or the next tile's x)."""
            # ---- qk: 4 blocks x 2 halves ----
            qA = qkpool.tile([96, TILE], bf16, tag="qA", name="qA")
            kA = qkpool.tile([96, TILE], bf16, tag="kA", name="kA")
            qB = qkpool.tile([96, TILE], bf16, tag="qB", name="qB")
            kB = qkpool.tile([96, TILE], bf16, tag="kB", name="kB")
            dsts = (qA, kA, qB, kB)
            for hf in range(2):
                rA, rB = fmA_half(ln1, hf), fmB_half(ln1, hf)
                for m in range(4):
                    ps = pp_h.tile([96, 512], f32, tag="ph", name="psqk")
                    nc.tensor.matmul(ps, sb["wqkA"][:, ds(96 * m, 96)], rA,
                                     start=True, stop=False)
                    nc.tensor.matmul(ps, sb["wqkB"][ds(64, 64), ds(96 * m, 96)], rB,
                                     start=False, stop=True)
                    dst = dsts[m][:, ts(hf, 512)]
                    if not biases_zero:
                        nc.scalar.activation(dst, ps, AF.Identity,
                                             bias=sb["bqk"][:, m:m + 1])
                    else:
                        nc.vector.tensor_copy(dst, ps)

            if _STOP_PHASE == "qk":
                return ln1
            # ---- scores + softmax + v, per sub ----
            v_tm = apool.tile([128, NSUB, D], bf16, tag="vtm", name="v_tm")
            pkms = []
            for s in range(NSUB):
                # one PSUM tile per PE tile-row: heads (hh, hh+3) share row
                # 32*hh; independent accumulation groups at different rows in
                # one PSUM tile lock up the device
                scR = [pp_sc.tile([128, 2, 128], f32, tag="sc", name=f"scR{r}")
                       for r in range(3)]
                for hd in range(HEADS):
                    grp, hh = hd // 3, hd % 3
                    qsrc = (qA, qB)[grp]
                    ksrc = (kA, kB)[grp]
                    nc.tensor.matmul(scR[hh][:, grp],
                                     qsrc[ds(32 * hh, 32), ts(s, 128)],
                                     ksrc[ds(32 * hh, 32), ts(s, 128)],
                                     start=True, stop=True)
                if _STOP_PHASE == "sconly":
                    continue
                # v for this sub (PE filler between score groups)
                psv = pp_sm.tile([128, D], f32, tag="sm", name="psv")
                nc.tensor.matmul(psv, ln1["fmA"][:, s, :], sb["wvA"],
                                 start=True, stop=False)
                nc.tensor.matmul(psv, ln1["fmB"][ds(64, 64), s, :], sb["wvB"][ds(64, 64)],
                                 start=False, stop=True)
                nc.vector.tensor_copy(v_tm[:, s], psv)
                if _STOP_PHASE == "sv":
                    continue
                # softmax (diagonal 64-blocks only)
                E = apool.tile([128, HEADS, 128], bf16, tag="E", name="E")
                sums = stpool.tile([128, HEADS], f32, tag="sm", name="sums")
                rsum = stpool.tile([128, HEADS], f32, tag="rs", name="rsum")
                for hh in range(3):
                    for half in range(2):
                        ho = half * 64
                        ebase = E[ds(ho, 64)]
                        esl = bass.AP(
                            tensor=ebase.tensor,
                            offset=ebase.offset + hh * 128 + ho,
                            ap=[ebase.ap[0], [3 * 128, 2], [1, 64]])
                        nc.scalar.activation(
                            esl, scR[hh][ds(ho, 64), :, ho:ho + 64], AF.Exp)
                if _STOP_PHASE == "exp":
                    continue
                nc.vector.reduce_sum(sums[0:64], E[0:64, :, 0:64],
                                     axis=mybir.AxisListType.X)
                nc.vector.reduce_sum(sums[ds(64, 64)], E[ds(64, 64), :, 64:128],
                                     axis=mybir.AxisListType.X)
                nc.vector.reciprocal(rsum, sums)
                if _STOP_PHASE == "sums":
                    continue
                P = p_bufs[pctr[0] % NPBUF]
                pctr[0] += 1
                for half in range(2):
                    hs = ds(half * 64, 64)
                    rs_half = rsum[ds(half * 64, 64)]
                    nc.gpsimd.tensor_tensor(
                        out=P[hs, :, hs], in0=E[hs, :, hs],
                        in1=bcast(rs_half, 64), op=OP.mult)
                if _STOP_PHASE == "norm":
                    continue
                pkm = apool.tile([128, HEADS, 128], bf16, tag="pkm", name="pkm")
                nc.sync.dma_start_transpose(
                    out=pkm, in_=P.rearrange("p a b -> p (a b)"))
                pkms.append(pkm)

            if _STOP_PHASE in ("sconly", "sconly2", "sv", "exp", "sums", "norm", "scores"):
                return ln1
            # next-tile LN1 depends only on the next x DMA: emit its whole
            # chain here so it overlaps this layer's attention tail + MLP
            ln_early = None
            if nxt_is_tile and x_nxt is not None:
                ln_early = ln_tiles()
                prefetch(AF.Sqrt)
                for s2 in range(NSUB):
                    ln_stats(ln_early, x_nxt, s2)
                ln_finish_half(ln_early, x_nxt, 0)
                ln_finish_half(ln_early, x_nxt, 1)
            elif biases_zero:
                prefetch(AF.Sqrt)

            # ---- O + out-copies + proj + residual + LN2 stats, per sub ----
            ln2 = ln_tiles()
            ofm0 = apool.tile([96, TILE], bf16, tag="ofm0", name="ofm0")
            ofm1 = apool.tile([96, TILE], bf16, tag="ofm1", name="ofm1")
            for s in range(NSUB):
                oP = pp_sm.tile([96, 256], f32, tag="sm", name="oP")
                for hd in range(HEADS):
                    grp, hh = hd // 3, hd % 3
                    nc.tensor.matmul(oP[ds(32 * hh, 32), ts(grp, 128)],
                                     v_tm[:, s, ts(hd, 32)], pkms[s][:, hd],
                                     start=True, stop=True,
                                     tile_position=(0, 32 * hh))
                if not biases_zero:
                    nc.scalar.activation(ofm0[:, ts(s, 128)], oP[:, 0:128],
                                         AF.Identity, bias=sb["bv"][:, 0:1])
                    nc.scalar.activation(ofm1[:, ts(s, 128)], oP[:, 128:256],
                                         AF.Identity, bias=sb["bv"][:, 1:2])
                else:
                    nc.vector.tensor_copy(ofm0[:, ts(s, 128)], oP[:, 0:128])
                    nc.vector.tensor_copy(ofm1[:, ts(s, 128)], oP[:, 128:256])
                psp = pp_sm.tile([128, D], f32, tag="sm", name="psp")
                nc.tensor.matmul(psp, ofm0[:, ts(s, 128)], sb["wpA"],
                                 start=True, stop=False)
                nc.tensor.matmul(psp, ofm1[:, ts(s, 128)], sb["wpB"],
                                 start=False, stop=True)
                nc.vector.tensor_add(x_t[:, s, 0:D], x_t[:, s, 0:D], psp)
                if not biases_zero:
                    nc.vector.tensor_add(x_t[:, s, 0:D], x_t[:, s, 0:D],
                                         sb["bp"])
                ln_stats(ln2, x_t, s)
                if s == 3:
                    if not biases_zero or (nxt_is_tile and x_nxt is not None):
                        prefetch(AF.Sqrt)
                    ln_finish_half(ln2, x_t, 0)
                elif s == 7:
                    ln_finish_half(ln2, x_t, 1)
                    prefetch(AF.Gelu_apprx_tanh)

            if _STOP_PHASE == "attn":
                return ln2
            # ---- MLP: fc1+gelu both halves, then fc2 + residual ----
            hfms = []
            for hf in range(2):
                rA, rB = fmA_half(ln2, hf), fmB_half(ln2, hf)
                hfm = mpool.tile([128, 6, 512], bf16, tag="hfm", name="hfm")
                for m in range(6):
                    ps = pp_h.tile([128, 512], f32, tag="ph", name="psh")
                    nc.tensor.matmul(ps, sb["w1A"][:, ts(m, 128)], rA,
                                     start=True, stop=False)
                    nc.tensor.matmul(ps, sb["w1B"][ds(64, 64), ts(m, 128)], rB,
                                     start=False, stop=True)
                    if biases_zero:
                        nc.scalar.activation(hfm[:, m], ps, AF.Gelu_apprx_tanh)
                    else:
                        nc.scalar.activation(hfm[:, m], ps, AF.Gelu_apprx_tanh,
                                             bias=sb["b1"][:, m:m + 1])
                hfms.append(hfm)
            ln_n = ln_early if (nxt_is_tile and ln_early is not None) else ln_tiles()
            for hf in range(2):
                for si in range(4):
                    s = 4 * hf + si
                    ps = pp_sm.tile([128, D], f32, tag="sm", name="psf2")
                    for m in range(6):
                        nc.tensor.matmul(ps, hfms[hf][:, m, ts(si, 128)],
                                         sb["w2m"][:, m],
                                         start=(m == 0), stop=(m == 5))
                    nc.vector.tensor_add(x_t[:, s, 0:D], x_t[:, s, 0:D], ps)
                    if not biases_zero:
                        nc.vector.tensor_add(x_t[:, s, 0:D], x_t[:, s, 0:D],
                                             sb["b2"])
                    if x_nxt is None or nxt_is_tile:
                        continue
                    ln_stats(ln_n, x_t, s)
                if x_nxt is None or nxt_is_tile:
                    continue
                if hf == 0:
                    prefetch(AF.Sqrt)
                    ln_finish_half(ln_n, x_t, 0)
                else:
                    ln_finish_half(ln_n, x_t, 1)
                    prefetch(AF.Exp)
            if nxt_is_tile:
                prefetch(AF.Exp)
            return ln_n

        # ======================================================================
        def load_tile(it):
            xt = xpool.tile([128, NSUB, D], f32, tag="x", name="x_t")
            nc.sync.dma_start(
                out=xt, in_=x_d[ts(it, TILE)].rearrange("(s p) f -> p s f",
                                                        p=128))
            return xt

        x_cur = load_tile(0)
        ln = ln_tiles()
        prefetch(AF.Sqrt)
        for s in range(4):
            ln_stats(ln, x_cur, s)
        ln_finish_half(ln, x_cur, 0)
        for s in range(4, 8):
            ln_stats(ln, x_cur, s)
        ln_finish_half(ln, x_cur, 1)
        prefetch(AF.Exp)

        for it in range(ntiles):
            x_nxt = load_tile(it + 1) if it + 1 < ntiles else None
            # layer 0: next LN is layer 1's LN1 on this x_t
            ln = emit_layer(W[0], x_cur, ln, x_cur, nxt_is_tile=False)
            # layer 1: next LN is the next tile's LN1 on x_nxt
            ln = emit_layer(W[1], x_cur, ln, x_nxt, nxt_is_tile=True)
            nc.sync.dma_start(
                out=y_d[ts(it, TILE)].rearrange("(s p) f -> p s f", p=128),
                in_=x_cur[:, :, 0:D])
            x_cur = x_nxt

    nc.compile()
    _COMPILED[key] = nc
    return nc


def _ensure_ntff_hook():
    """The image's antenv package lacks axon_hooks; synthesize it and install
    the ctypes-based NTFF profile hook from trn_agent_boot (test-only path)."""
    import sys, types
    if "antenv.axon_hooks" in sys.modules:
        return True
    try:
        mod = types.ModuleType("antenv.axon_hooks")
        state = {}
        mod.set_axon_ntff_profile_hook = lambda h: state.__setitem__("h", h)
        mod.get_axon_ntff_profile_hook = lambda: state.get("h")
        sys.modules["antenv.axon_hooks"] = mod
        import antenv
        antenv.axon_hooks = mod
        from trn_agent_boot.trn_boot import _ntff_profile_via_ctypes
        mod.set_axon_ntff_profile_hook(
            _ntff_profile_via_ctypes("/opt/axon/libaxon_pjrt.so"))
        return True
    except Exception as e:  # pragma: no cover
        print(f"NTFF hook shim failed: {e}")
        return False


def _run(inputs, trace=False):
    """Shard, execute on 8 cores, gather. Returns (y_full, exec_time_ns)."""
    from concourse.bass_utils import run_bass_kernel_spmd

    if trace:
        trace = _ensure_ntff_hook()

    layers = _fold_weights(inputs)
    bz = all(
        not np.any(np.asarray(d[k], np.float32))
        for d in layers for k in d
        if k.startswith(("bp", "b2", "bqk", "bv", "b1")))
    nc = _build_nc(biases_zero=bz)

    x = np.asarray(inputs["x"], np.float32)
    pos = np.asarray(inputs["pos"], np.float32)
    w = int(np.asarray(inputs["w"]))
    order = _scanline_order(pos, w)
    x_ord = np.take_along_axis(x, order[..., None], axis=1)
    shards = np.ascontiguousarray(x_ord.reshape(NCORES, T, D))

    wmap = {}
    for d in layers:
        wmap.update({k: np.ascontiguousarray(v) for k, v in d.items()})

    in_maps = [{"x": shards[c], **wmap} for c in range(NCORES)]
    res = run_bass_kernel_spmd(nc, in_maps, core_ids=list(range(NCORES)),
                               trace=trace)
    y_ord = np.stack([res.results[c]["y"] for c in range(NCORES)])
    y_ord = y_ord.reshape(B, N, D)
    y = np.empty_like(y_ord)
    np.put_along_axis(y, order[..., None], y_ord, axis=1)
    return y.astype(np.float32), res.exec_time_ns


def kernel(**inputs):
    y, _ = _run(inputs, trace=False)
    return y
